# revision 13
# baseline (speedup 1.0000x reference)
"""Batched 3-layer GCN (nn_BatchGCN) on 8 TRN2 NeuronCores — one graph per core.

Math per graph, per layer:  h' = Ahat @ (h @ W.T + b),  Ahat = D^-1/2 A D^-1/2
(duplicate edges accumulate; relu between layers 1-2).  The symmetric
normalization factors node-wise, so only dense node-wise scaling is needed.

Device algorithm (per core). The scatter (segment-sum over edges) uses
TensorEngine one-hot matmuls in the "flipped" orientation: gathered messages
are the stationary operand (64 cols -> cheap LDWEIGHTS), the one-hot is the
moving operand, accumulating feat-major [64f x 64d] PSUM per destination
block; four consecutive blocks share one PSUM bank and drain with a single
wide DVE multiply. h stays feat-major [64, N] between layers so the next
layer's W matmul needs no transpose.

  - host relabels nodes (LPT bin-packing by in-degree) so every 64-node
    destination block has <= CB*128 incoming edge slots (CB=8 here, no tail
    waste); dummy slots carry row-offset 1000 -> all-zero one-hot column
  - degree pass: same one-hot structure vs an all-ones stationary gives deg
    feat-major; dinv = mask * 2*Dsqrt(max(deg,1)) (Dsqrt(x) = 0.5/sqrt(x))
  - per layer: z = W.T @ h on PE, (+bias)*dinv in one fused DVE op,
    PE-transpose to node-major, cast-copy to a bf16 stage, one contiguous DMA
    into a [N, 128]-row bf16 DRAM table (256B rows, cols 64-127 junk);
    gpsimd dma_gather fetches 256B messages per 4-block chunk, round-robined
    over 4 SWDGE queues so descriptor generation uses all 8 Q7 cores; drains
    are delayed one gather so PSUM waits never head-of-line-block the DVE
All matmul accumulation is f32 in PSUM; messages/one-hots are bf16.

Host-side work is index/layout marshaling only (permute/pad/bucket the
given arrays); all arithmetic on tensor values happens on-device.
"""
from dataclasses import dataclass

import heapq

import ml_dtypes
import numpy as np

import concourse.bacc as bacc
import concourse.mybir as mybir
import concourse.tile as tile
from concourse.bass import broadcast_tensor_aps
from concourse.bass_utils import run_bass_kernel_spmd
from concourse.library_config import mlp

B, NV, E, F = 8, 10000, 160000, 64
N = 10240          # padded node count
DW = 64            # destination block width
NB = N // DW       # 160 destination blocks
NBLK = N // 128    # 80 partition-major z blocks
CORES = list(range(8))
BF16 = ml_dtypes.bfloat16


@dataclass(frozen=True)
class _Cfg:
    CB: int        # chunks (of 128 edge slots) per 64-wide destination block
    GB: int = 4    # destination blocks per gather chunk
    layers: int = 3

    @property
    def epad(self):
        return NB * self.CB * 128

    @property
    def ngch(self):
        return NB // self.GB


def _build(cfg: _Cfg, trips: int = 0):
    CB, GB = cfg.CB, cfg.GB
    EPAD, NG = cfg.epad, cfg.ngch
    QC = GB * CB               # 128-slot chunks per gather / oh quad
    NIDX = QC * 128            # indices per gather
    GW = GB * DW               # dest columns per gather (256)
    NQ = N // 512              # z chunks

    nc = bacc.Bacc("TRN2", debug=False, num_swdge_queues=4)
    x_hbm = nc.dram_tensor("x_fm", [64, N], mybir.dt.float32, kind="ExternalInput")
    w_hbm = nc.dram_tensor("w_t", [128, cfg.layers * F], mybir.dt.float32, kind="ExternalInput")
    b_hbm = nc.dram_tensor("bias", [128, cfg.layers], mybir.dt.float32, kind="ExternalInput")
    i_hbm = nc.dram_tensor("ident", [64, 64], mybir.dt.float32, kind="ExternalInput")
    t_hbm = nc.dram_tensor("iota_t", [128, QC * DW], mybir.dt.bfloat16, kind="ExternalInput")
    r_hbm = nc.dram_tensor("rowoff", [128, EPAD // 128], mybir.dt.bfloat16, kind="ExternalInput")
    c_hbm = nc.dram_tensor("colr", [128, EPAD // 16], mybir.dt.int16, kind="ExternalInput")
    out_hbm = nc.dram_tensor("out_pm", [128, NBLK * F], mybir.dt.float32, kind="ExternalOutput")
    zdram = [nc.dram_tensor(f"zdram{i}", [N, 128], mybir.dt.bfloat16) for i in range(2)]

    with tile.TileContext(nc) as tc:
        with (
            tc.tile_pool(name="const", bufs=1) as cp,
            tc.tile_pool(name="state", bufs=1) as sp,
            tc.tile_pool(name="oh", bufs=4) as ohp,
            tc.tile_pool(name="msg", bufs=6) as mp,
            tc.tile_pool(name="zb", bufs=2) as zp,
            tc.tile_pool(name="tm", bufs=3) as tp,
            tc.tile_pool(name="pz", bufs=2, space="PSUM") as pz,
            tc.tile_pool(name="pt", bufs=2, space="PSUM") as pt,
            tc.tile_pool(name="pt3", bufs=1, space="PSUM") as pt3,
            tc.tile_pool(name="psc", bufs=3, space="PSUM") as psc,
        ):
            nc.gpsimd.load_library(mlp)

            wt = cp.tile([128, cfg.layers, F], mybir.dt.float32)
            nc.sync.dma_start(wt[:], w_hbm[:].rearrange("p (l f) -> p l f", l=cfg.layers))
            bs = cp.tile([128, cfg.layers], mybir.dt.float32)
            nc.sync.dma_start(bs[:], b_hbm[:])
            ident = cp.tile([64, 64], mybir.dt.float32)
            nc.sync.dma_start(ident[:], i_hbm[:])
            iota = cp.tile([128, QC * DW], mybir.dt.bfloat16)
            nc.sync.dma_start(iota[:], t_hbm[:])
            rowoff = cp.tile([128, EPAD // 128], mybir.dt.bfloat16)
            nc.sync.dma_start(rowoff[:], r_hbm[:])
            colr = cp.tile([128, EPAD // 16], mybir.dt.int16)
            nc.sync.dma_start(colr[:], c_hbm[:])
            ones = cp.tile([128, F], mybir.dt.bfloat16)
            nc.vector.memset(ones[:], 1.0)

            hAB = sp.tile([128, N], mybir.dt.float32, tag="hAB")
            dinv = sp.tile([64, N], mybir.dt.bfloat16, tag="dinv")
            zstage = sp.tile([128, NBLK, 128], mybir.dt.bfloat16, tag="zstage")
            stage = sp.tile([128, NBLK, F], mybir.dt.float32, tag="stage")
            # degree-phase scratch overlays zstage (first used after dinv done)
            scr = zstage[0:64].rearrange("p a b -> p (a b)")

            def onehot_quad(g):
                # oh[e, k*64+d] = (rowoff[e, g*QC+k] == d): one gather's blocks
                oh = ohp.tile([128, QC, DW], mybir.dt.bfloat16, tag="oh")
                ro3 = rowoff[:, g * QC:(g + 1) * QC].rearrange("p c -> p c ()")
                io3 = iota[:].rearrange("p (c d) -> p c d", c=QC)
                a, bb = broadcast_tensor_aps(io3, ro3)
                nc.vector.tensor_tensor(out=oh[:], in0=a, in1=bb,
                                        op=mybir.AluOpType.is_equal)
                return oh

            def body():
                nc.sync.dma_start(hAB[0:64, :], x_hbm[:])

                # ---- degree pass: deg (feat-replicated) into dinv ----
                for g in range(NG):
                    oh = onehot_quad(g)
                    ps = psc.tile([64, GB, DW], mybir.dt.float32, tag="psc")
                    for bb_ in range(GB):
                        for c in range(CB):
                            nc.tensor.matmul(
                                ps[:, bb_], ones[:], oh[:, bb_ * CB + c],
                                start=(c == 0), stop=(c == CB - 1))
                    nc.scalar.copy(dinv[:, g * GW:(g + 1) * GW], ps[:])
                # dinv = (deg>0) * max(deg,1)^-1/2 = (deg>0)/sqrt(deg)
                nc.vector.tensor_scalar(out=scr, in0=dinv[:], scalar1=0.5,
                                        scalar2=None, op0=mybir.AluOpType.is_gt)
                nc.vector.tensor_scalar(out=dinv[:], in0=dinv[:], scalar1=1.0,
                                        scalar2=None, op0=mybir.AluOpType.max)
                nc.scalar.activation(dinv[:], dinv[:],
                                     mybir.ActivationFunctionType.Abs_reciprocal_sqrt)
                with nc.allow_low_precision(reason="dinv kept bf16 by design"):
                    nc.vector.tensor_tensor(out=dinv[:], in0=dinv[:], in1=scr,
                                            op=mybir.AluOpType.mult)

                ha, hb = 0, 64
                for lay in range(cfg.layers):
                    zd = zdram[lay % 2]
                    # ---- z = (W.T @ h + b) * dinv; node-major bf16 stage ----
                    for q in range(NQ):
                        pzt = pz.tile([64, 512], mybir.dt.float32, tag="pz")
                        nc.tensor.matmul(
                            pzt[:], wt[ha:ha + 64, lay],
                            hAB[ha:ha + 64, q * 512:(q + 1) * 512],
                            start=True, stop=True)
                        zb = zp.tile([64, 512], mybir.dt.float32, tag="zb")
                        with nc.allow_low_precision(reason="z~ scaled to bf16 table"):
                            nc.vector.scalar_tensor_tensor(
                                out=zb[:], in0=pzt[:],
                                scalar=bs[0:64, lay:lay + 1],
                                in1=dinv[:, q * 512:(q + 1) * 512],
                                op0=mybir.AluOpType.add, op1=mybir.AluOpType.mult)
                        for j in range(4):
                            blk = 4 * q + j
                            ptt = pt.tile([128, 64], mybir.dt.float32, tag="pt")
                            nc.tensor.transpose(
                                ptt[:], zb[:, j * 128:(j + 1) * 128], ident[:])
                            nc.scalar.copy(zstage[:, blk, 0:64], ptt[:])
                    nc.sync.dma_start(
                        zd[:].rearrange("(p c) f -> p c f", p=128), zstage[:])

                    # ---- gather + segment-sum, drains delayed one gather ----
                    def drain(ps, g):
                        tmp = tp.tile([64, GW], mybir.dt.float32, tag="tmp")
                        with nc.allow_low_precision(reason="h scaled by bf16 dinv"):
                            nc.vector.tensor_tensor(
                                out=tmp[:], in0=ps[:],
                                in1=dinv[:, g * GW:(g + 1) * GW],
                                op=mybir.AluOpType.mult)
                        if lay < cfg.layers - 1:
                            nc.scalar.activation(
                                hAB[hb:hb + 64, g * GW:(g + 1) * GW], tmp[:],
                                mybir.ActivationFunctionType.Relu)
                        else:
                            for j in range(GB):
                                b = g * GB + j
                                p3 = pt3.tile([64, 64], mybir.dt.float32, tag="pt3")
                                nc.tensor.transpose(
                                    p3[:], tmp[:, j * 64:(j + 1) * 64], ident[:])
                                po = (b & 1) * 64
                                nc.scalar.copy(stage[po:po + 64, b >> 1], p3[:])

                    prev = None
                    for g in range(NG):
                        msgs = mp.tile([128, QC, 128], mybir.dt.bfloat16, tag="msgs")
                        nc.gpsimd.dma_gather(
                            msgs[:], zd[:],
                            colr[:, g * (NIDX // 16):(g + 1) * (NIDX // 16)],
                            NIDX, NIDX, 128, single_packet=False,
                            queue_num=g % 4)
                        oh = onehot_quad(g)
                        ps = psc.tile([64, GB, DW], mybir.dt.float32, tag="psc")
                        for bb_ in range(GB):
                            for c in range(CB):
                                ci = bb_ * CB + c
                                nc.tensor.matmul(
                                    ps[:, bb_], msgs[:, ci, 0:64], oh[:, ci],
                                    start=(c == 0), stop=(c == CB - 1))
                        if prev is not None:
                            drain(*prev)
                        prev = (ps, g)
                    drain(*prev)
                    ha, hb = hb, ha
                nc.sync.dma_start(
                    out_hbm[:].rearrange("p (c f) -> p c f", c=NBLK), stage[:])

            if trips > 0:
                with tc.For_i(0, trips):
                    body()
            else:
                body()

    nc.compile()
    return nc


def _balance(edge_index):
    """Relabel nodes so each 64-node dest block gets <= CB*128 edge slots.
    Returns (perm old->new, max block load)."""
    row = np.asarray(edge_index[:, 0], np.int64)
    deg = np.bincount(row, minlength=NV)
    order = np.argsort(-deg, kind="stable")
    loads = [(0, b) for b in range(NB)]
    heapq.heapify(loads)
    slots = np.zeros(NB, np.int64)
    perm = np.empty(NV, np.int64)
    for nid in order:
        load, b = heapq.heappop(loads)
        perm[nid] = b * DW + slots[b]
        slots[b] += 1
        if slots[b] < DW:
            heapq.heappush(loads, (load + deg[nid], b))
    blkload = np.bincount(perm[row] >> 6, minlength=NB)
    return perm, int(blkload.max())


def _prep_inputs(cfg: _Cfg, x, edge_index, Ws, bs_, perm):
    """Index/layout marshaling for one graph (no value arithmetic)."""
    CB, EPAD = cfg.CB, cfg.epad
    row = perm[np.asarray(edge_index[:, 0], np.int64)]
    col = perm[np.asarray(edge_index[:, 1], np.int64)]
    blk = row >> 6
    order = np.argsort(blk, kind="stable")
    counts = np.bincount(blk, minlength=NB)
    assert counts.max() <= CB * 128, f"block overflow: {counts.max()}"
    starts = np.cumsum(counts) - counts
    base = np.repeat(np.arange(NB) * CB * 128, counts)
    within = np.arange(len(row)) - np.repeat(starts, counts)
    slots = base + within
    rowoff = np.full(EPAD, 1000.0, np.float32)
    colv = np.zeros(EPAD, np.int64)
    rowoff[slots] = (row & 63)[order]
    colv[slots] = col[order]
    # node id -> row of the partition-major z~ DRAM table
    colr = ((colv & 127) * NBLK + (colv >> 7)).astype(np.int16)

    def wrap16(a):
        w = a.reshape(-1, 16).T
        return np.tile(w, (8, 1))

    rowoff_t = np.ascontiguousarray(rowoff.reshape(-1, 128).T).astype(BF16)
    colr_t = wrap16(colr)

    x_fm = np.zeros((64, N), np.float32)
    x_fm[:, perm] = np.asarray(x, np.float32).T

    w_t = np.zeros((128, len(Ws), F), np.float32)
    bias = np.zeros((128, len(Ws)), np.float32)
    for l, (W, b) in enumerate(zip(Ws, bs_)):
        w_t[:64, l] = np.asarray(W, np.float32).T
        w_t[64:, l] = np.asarray(W, np.float32).T
        bias[:64, l] = np.asarray(b, np.float32)
        bias[64:, l] = np.asarray(b, np.float32)

    return {
        "x_fm": x_fm,
        "w_t": np.ascontiguousarray(w_t.reshape(128, -1)),
        "bias": bias,
        "ident": np.eye(64, dtype=np.float32),
        "iota_t": np.tile(np.tile(np.arange(DW, dtype=np.float32), cfg.GB * CB),
                          (128, 1)).astype(BF16),
        "rowoff": rowoff_t,
        "colr": colr_t,
    }


def _unpack_output(cfg: _Cfg, out_pm, perm):
    o = out_pm.reshape(128, NBLK, F).transpose(1, 0, 2).reshape(N, F)
    return o[perm]


def kernel(x, edge_index, W1, b1, W2, b2, W3, b3):
    x = np.asarray(x)
    edge_index = np.asarray(edge_index)
    Ws = [np.asarray(W1), np.asarray(W2), np.asarray(W3)]
    bs_ = [np.asarray(b1), np.asarray(b2), np.asarray(b3)]
    nb = x.shape[0]
    assert x.shape == (B, NV, F) and edge_index.shape == (B, E, 2)

    perms, maxloads = zip(*(_balance(edge_index[g]) for g in range(nb)))
    CB = max(8, -(-max(maxloads) // 128))
    cfg = _Cfg(CB=CB)

    in_maps = [_prep_inputs(cfg, x[g], edge_index[g], Ws, bs_, perms[g])
               for g in range(nb)]
    nc = _build(cfg)
    try:
        res = run_bass_kernel_spmd(nc, in_maps, CORES).results
    except Exception:
        # transient NRT device wedge recovers on a fresh attempt
        res = run_bass_kernel_spmd(nc, in_maps, CORES).results
    out = np.stack([_unpack_output(cfg, res[g]["out_pm"], perms[g])
                    for g in range(nb)])
    return out.astype(np.float32)
